# revision 19
# baseline (speedup 1.0000x reference)
"""Trainium2 Bass kernel: multi-head attention with per-head QK LayerNorm.

Problem shapes: B=2, S=2048, D=1024, H=16 heads, head_dim=64, fp32 in/out.

Sharding (8 cores): core c handles batch b = c//4 and head-group g = c%4
(4 heads = 256 qkv dims). Each core computes its heads' attention and a
partial out-projection; the host sums the 4 partials per batch entry
(tensor-parallel all-reduce done on host at unshard time) and adds o_b.

Key algebraic restructurings (all exact, modulo fp rounding):
  - LN mean subtraction and gain g are linear => folded into q_w/k_w (and
    biases) on the host.  Kernel computes qg = g*(q - mean(q)) directly.
  - LN variance = sum(w_d * qg_d^2) with w_d = 1/(64*g_d^2): computed on
    device from qg^2 via small block-diagonal stats matmuls.
  - rstd = exp(-0.5*ln(var)) on the ACT engine: Ln and Exp share ONE
    table set (natural_log_exp_and_others), so no table switching, and
    the DVE is freed of the old ~4.6us/chain Newton rsqrt chains.
  - rstd_q is folded into qT columns and tau*rstd_k into kT columns
    (via PE onesel broadcast matmuls), so softmax is a bare exp() of the
    raw scores.  Scores are computed TRANSPOSED: [kv on partitions,
    q on free], which feeds AV directly with no PE transposes.
  - softmax max-subtraction is skipped: post-LN rows have norm 8, so
    |scores| <= 8 and exp() stays in range.
  - sum(exp) over kv falls out of the AV matmul via a ones-column
    appended to V.  Normalization happens on attT eviction.

Perf notes (v7; baseline v2 was 272us, ACT-exp-paced with a 70us serial
startup):
  - Steady state is PE-work-bound (~9.8/10 tensor busy): per group the PE
    does a QK pair (~315ns, concurrent on row tiles 0/64), 2 AV mms
    (~228ns each) plus woven filler work (projection chains, v-chains,
    out-projections).  Every 3rd group's exp runs on the DVE as a
    one-instruction Schraudolph (int16(x*1477.32+15316) bitcast f16,
    ~1.8% RMS on ~1/3 of weights), which breaks the ACT 1147ns/group
    pacing floor; ACT also computes the rstds and the av-bank evictions
    and stays ~25% under the PE.
  - Startup: a ~8.5us PE spin on scratch data holds the HAM clock gate at
    2.4GHz (otherwise everything before ~16us runs at k=4 half clock).
    Input DMA issue is spread over the sync/gpsimd/scalar queues
    (~600ns/descriptor per queue); xT halves stream on sync and gpsimd
    concurrently.  Only k(c0,sb0)+q(c0,sb0) run before the exp stream;
    all other chains weave in as fillers, gated by emission order so
    kT/qTs slices exist before the QK that reads them.
  - Chains are split A (8 proj mms + evict + square) / B1 (stats mm +
    ACT ln + ACT exp) / B2 (PE onesel bcast + DVE scale-mul), emitted at
    consecutive filler slots so no engine queue stalls on a cross-engine
    dependency.
  - Block-boundary norm: the AV accumulators are single-buffered, so the
    [65,512] accumulator is evicted to SBUF by ONE ACT copy (banks free
    immediately), the two heads' norm chains are emitted stage-major to
    pipeline ACT/DVE/gpsimd, and the next block's j<3 AVs are deferred 3
    extra slots.  The last block broadcasts the recip rows on the PE
    instead of gpsimd and the tail out-projection pre-runs its c0
    matmuls into the freed qk banks while the last AVs stream.
  - PSUM: scores 2x[128,2,512] (4 banks) + AV accum 2 + acc pool
    (proj/stats/out-proj) 2 = 8 banks exactly.
"""

import os
import sys

import numpy as np

for _p in ("/opt/trn_rl_repo",):
    if _p not in sys.path:
        sys.path.append(_p)

# ---- problem constants (hardcoded; kernel.py must be self-contained) ----
B, S, D, H, HD = 2, 2048, 1024, 16, 64
EPS = 1e-5
NCORES = 8
GPC = 4            # cores per batch entry (head-groups)
HL = H // GPC      # 4 local heads
DL = HL * HD       # 256 local qkv dims
P = 128
KC = D // P        # 8 contraction chunks for projections
CL = DL // P       # 2 local-dim partition chunks (head pairs)
SB = 512           # free-dim block (= one PSUM bank of fp32)
NSB = S // SB      # 4 blocks
NKV = S // P       # 16 kv chunks
STW = 33           # stats lhsT cols: head vars at partitions 0 and 32

_CACHE = {}


def _build_nc():
    """Build the (single, SPMD-shared) Bass program for one core."""
    import concourse.bass as bass
    import concourse.mybir as mybir
    import concourse.tile as tile
    from concourse import bacc
    from concourse.dve_ops import RECIPROCAL_APPROX_FAST, RECIP_APPROX_FAST_CONSTS

    f32 = mybir.dt.float32
    f16 = mybir.dt.float16
    AF = mybir.ActivationFunctionType
    rc = RECIP_APPROX_FAST_CONSTS

    def recip(nc, out, in_):
        # ~51-ULP reciprocal in a single DVE pass (vs ~6 cyc/elem exact).
        return nc.vector._custom_dve(
            RECIPROCAL_APPROX_FAST, out=out, in0=in_,
            s0=rc["s0"], s1=rc["s1"], imm2=rc["imm2"],
        )

    class _Bacc(bacc.Bacc):
        def insert_act_table_loads(self):
            """As Bacc's, but Exp/Ln are hidden from every table set except
            natural_log_exp_and_others, so the (greedy, first-match) load
            pass resolves both to ONE set instead of ping-ponging between
            exp_and_others and natural_log (~2.7us per switch, 29 loads).
            Canonical set order/ids are preserved."""
            import bass_rust as _bass_rust
            from concourse.hw_specs import get_activation_tables

            if not any(isinstance(i, mybir.InstActivation)
                       for b in self.main_func.blocks
                       for i in b.instructions):
                return
            joint = {AF.Exp, AF.Ln}
            tables = [(n, s if n == "natural_log_exp_and_others" else s - joint)
                      for n, s in get_activation_tables(self.m.arch).items()]
            _bass_rust.insert_act_table_loads(self, tables)

    nc = _Bacc(trn_type="TRN2")

    xT_d = nc.dram_tensor("xT", [KC, P, S], f16, kind="ExternalInput")
    wqT_d = nc.dram_tensor("wqT", [KC, P, DL], f16, kind="ExternalInput")
    wkT_d = nc.dram_tensor("wkT", [KC, P, DL], f16, kind="ExternalInput")
    wvT_d = nc.dram_tensor("wvT", [KC, P, DL], f16, kind="ExternalInput")
    woT_d = nc.dram_tensor("woT", [CL, P, D], f16, kind="ExternalInput")
    qb_d = nc.dram_tensor("qb", [CL, P, 1], f32, kind="ExternalInput")
    kb_d = nc.dram_tensor("kb", [CL, P, 1], f32, kind="ExternalInput")
    vb_d = nc.dram_tensor("vb", [1, DL], f32, kind="ExternalInput")
    wsq_d = nc.dram_tensor("wsq", [CL, P, STW], f16, kind="ExternalInput")
    wsk_d = nc.dram_tensor("wsk", [CL, P, STW], f16, kind="ExternalInput")
    out_d = nc.dram_tensor("out", [NKV, P, D], f16, kind="ExternalOutput")

    with tile.TileContext(nc) as tc:
        with tc.tile_pool(name="big", bufs=1) as big:
            # ---- persistent SBUF; DMA issue split over 4 engine queues so
            # the ~600ns/descriptor issue cost doesn't serialize the 6MB
            # input stream.  Per-queue issue order = need order.
            xt = [big.tile([P, S], f16, name=f"xt{k}") for k in range(KC)]
            wk_sb = [big.tile([P, DL], f16, name=f"wk{k}") for k in range(KC)]
            wq_sb = [big.tile([P, DL], f16, name=f"wq{k}") for k in range(KC)]
            wv_sb = [big.tile([P, DL], f16, name=f"wv{k}") for k in range(KC)]
            kb_sb = big.tile([P, CL, 1], f32, name="kb_sb")
            qb_sb = big.tile([P, CL, 1], f32, name="qb_sb")
            wsq_sb = big.tile([P, CL, STW], f16, name="wsq_sb")
            wsk_sb = big.tile([P, CL, STW], f16, name="wsk_sb")
            vb_bc = big.tile([P, DL], f32, name="vb_bc")
            wo_sb = big.tile([P, CL, D], f16, name="wo_sb")

            # sync: xT sb0+sb1 (first chains + v0-7), then late-needed
            # weights (wv half for ~25us, wo for ~90us)
            for sb in range(2):
                for k in range(KC):
                    nc.sync.dma_start(xt[k][:, sb * SB:(sb + 1) * SB],
                                      xT_d[k, :, sb * SB:(sb + 1) * SB])
            for k in range(KC // 2, KC):
                nc.sync.dma_start(wv_sb[k], wvT_d[k])
            for c in range(CL):
                nc.sync.dma_start(wo_sb[:, c, :], woT_d[c])
            # gpsimd: wk first (gates the first chain's mms at ~4us), then
            # the rest of the k-side, half of wv, and xT sb2+sb3 (needed
            # from ~group 8 of the stream on; two queues deliver xT
            # concurrently instead of one 23us serial issue stream)
            for k in range(KC):
                nc.gpsimd.dma_start(wk_sb[k], wkT_d[k])
            for k in range(KC):
                nc.gpsimd.dma_start(wq_sb[k], wqT_d[k])
            for c in range(CL):
                nc.gpsimd.dma_start(kb_sb[:, c, :], kb_d[c])
                nc.gpsimd.dma_start(wsk_sb[:, c, :], wsk_d[c])
            for k in range(KC // 2):
                nc.gpsimd.dma_start(wv_sb[k], wvT_d[k])
            nc.gpsimd.dma_start(vb_bc, vb_d[:].to_broadcast((P, DL)))
            for sb in range(2, NSB):
                for k in range(KC):
                    nc.gpsimd.dma_start(xt[k][:, sb * SB:(sb + 1) * SB],
                                        xT_d[k, :, sb * SB:(sb + 1) * SB])
            # scalar: only the 4 tiny q-side bias/stat tiles (~2.5us of
            # issue) so the ACT table load runs at ~3us and the first rstd
            # ln/exp isn't stuck behind DMA issues (wq rides gpsimd)
            for c in range(CL):
                nc.scalar.dma_start(qb_sb[:, c, :], qb_d[c])
                nc.scalar.dma_start(wsq_sb[:, c, :], wsq_d[c])

            # scratch for the startup PE spin (memset so the garbage-free
            # reads can't hit denormal/NaN corner paths)
            spin_sb = big.tile([P, SB], f16, name="spin_sb")
            nc.vector.memset(spin_sb, 1.0)

            kT_sb = big.tile([P, CL, S], f16, name="kT_sb")
            qTs_sb = big.tile([P, CL, S], f16, name="qTs_sb")
            vaug_sb = big.tile([P, NKV, HL, HD + 1], f16, name="vaug_sb")
            attT_sb = big.tile([P, CL, S], f16, name="attT_sb")
            nc.vector.memset(vaug_sb[:, :, :, HD:HD + 1], 1.0)
            # onesel broadcasts rstd rows (partitions 0 and 32) to the 128
            # qkv partitions via a matmul: col m reads partition 0 (m<64)
            # or partition 32 (m>=64).
            onesel = big.tile([STW, P], f16, name="onesel")
            nc.vector.memset(onesel, 0.0)
            nc.vector.memset(onesel[0:1, 0:HD], 1.0)
            nc.vector.memset(onesel[32:33, HD:P], 1.0)
            # per-side scv*EPS bias columns for the Ln rstd path (the
            # activation bias operand must be a [P,1] SBUF AP)
            eps_k = big.tile([STW, 1], f32, name="eps_k")
            eps_q = big.tile([STW, 1], f32, name="eps_q")
            nc.vector.memset(eps_k, 64.0 * EPS)
            nc.vector.memset(eps_q, EPS)

            with tc.tile_pool(name="acc", bufs=2, space="PSUM") as acc, \
                 tc.tile_pool(name="qk", bufs=2, space="PSUM") as qk, \
                 tc.tile_pool(name="av", bufs=1, space="PSUM") as avp, \
                 tc.tile_pool(name="sq", bufs=5) as sq, \
                 tc.tile_pool(name="ev", bufs=4) as ev, \
                 tc.tile_pool(name="ex", bufs=11) as exp_pool:

                SIDES = {
                    "k": (wk_sb, kb_sb, wsk_sb, kT_sb, 64.0, eps_k),
                    "q": (wq_sb, qb_sb, wsq_sb, qTs_sb, 1.0, eps_q),
                }

                def chain_items(side, c, sb):
                    """q/k projection chain, split A/B1/B2 so no engine
                    queue stalls on a cross-engine dependency (parts are
                    emitted at consecutive filler slots)."""
                    wlist, bcol, wst, dst, scv, epsc = SIDES[side]
                    st = {}

                    def part_a():
                        ph = acc.tile([P, SB], f32, name="ph", tag="acc")
                        for k in range(KC):
                            nc.tensor.matmul(
                                ph, wlist[k][:, c * P:(c + 1) * P],
                                xt[k][:, sb * SB:(sb + 1) * SB],
                                start=(k == 0), stop=(k == KC - 1),
                            )
                        tr = sq.tile([P, SB], f16, name="tr_t")
                        nc.vector.tensor_scalar_add(tr, ph, bcol[:, c, :])
                        qsq = sq.tile([P, SB], f16, name="sq_t")
                        nc.vector.tensor_mul(qsq, tr, tr)
                        st["tr"], st["qsq"] = tr, qsq

                    def part_b1():
                        # stats lhsT has 33 cols: head0 var -> partition 0,
                        # head1 var -> partition 32 (engines may only access
                        # partition bases aligned to 32).
                        stp = acc.tile([STW, SB], f32, name="stp", tag="acc")
                        nc.tensor.matmul(stp, wst[:, c, :], st["qsq"],
                                         start=True, stop=True)
                        # rstd = (scv*var + scv*eps)^-0.5 as exp(-0.5*ln(z)):
                        # Ln and Exp live in one ACT table set, and the
                        # scv*z+eps affine rides Ln's free scale/bias input.
                        zl = ev.tile([STW, SB], f32, name="zl", bufs=3)
                        nc.scalar.activation(zl, stp, AF.Ln,
                                             bias=epsc[:, :], scale=scv)
                        rr = ev.tile([STW, SB], f16, name="rr", bufs=3)
                        nc.scalar.activation(rr, zl, AF.Exp, scale=-0.5)
                        st["rr"] = rr

                    def part_b2():
                        # broadcast rstd rows to all 128 partitions on the PE
                        # (partition_broadcast with out base 64 is broken on
                        # HW; SBUF->SBUF broadcast DMA has multi-us latency).
                        qsc = acc.tile([P, SB], f32, name="qsc", tag="acc")
                        nc.tensor.matmul(qsc, onesel, st["rr"],
                                         start=True, stop=True)
                        nc.vector.tensor_mul(
                            dst[:, c, sb * SB:(sb + 1) * SB], st["tr"], qsc)

                    return [("chain", part_a), ("chain", part_b1),
                            ("chain", part_b2)]

                def v_item(mc):
                    def f():
                        pv = acc.tile([P, SB], f32, name="pv",
                                      tag="acc")[:, :DL]
                        for k in range(KC):
                            nc.tensor.matmul(
                                pv, xt[k][:, mc * P:(mc + 1) * P], wv_sb[k],
                                start=(k == 0), stop=(k == KC - 1),
                            )
                        nc.vector.tensor_add(
                            vaug_sb[:, mc, :, 0:HD],
                            pv.rearrange("p (h d) -> p h d", d=HD),
                            vb_bc.rearrange("p (h d) -> p h d", d=HD),
                        )
                    return [("v", f)]

                def op_item(m, nb, use_qk=False, dma_eng=None):
                    def f():
                        if use_qk:
                            pon = qk.tile([P, 2, SB], f32,
                                          name="qk_t")[:, 0, :]
                        else:
                            pon = acc.tile([P, SB], f32, name="pon",
                                           tag="acc")
                        for c in range(CL):
                            nc.tensor.matmul(
                                pon, attT_sb[:, c, m * P:(m + 1) * P],
                                wo_sb[:, c, nb * SB:(nb + 1) * SB],
                                start=(c == 0), stop=(c == CL - 1),
                            )
                        osb = ev.tile([P, SB], f16, name="osb")
                        nc.vector.tensor_copy(osb, pon)
                        eng = dma_eng if dma_eng is not None else nc.sync
                        eng.dma_start(
                            out_d[m, :, nb * SB:(nb + 1) * SB], osb)
                    return [("op", f)]

                # ---- PE spin: ~8.5us of dependency-free matmuls so the
                # HAM clock gate holds 2.4GHz while the input DMA streams
                # in (idle PE -> k=4 -> the first chains run at half speed)
                spin_ps = acc.tile([P, SB], f32, name="spin_ps", tag="acc")
                for _ in range(36):
                    nc.tensor.matmul(spin_ps, spin_sb[:, 0:P], spin_sb,
                                     start=True, stop=True)

                # ---- upfront: ONLY k(c0,sb0) + q(c0,sb0); everything else
                # weaves into the exp stream.  The first QK (block 0, j=0)
                # needs just these two chains' kT/qTs slices.  A parts are
                # back-to-back so the q chain's 8 proj mms overlap the k
                # chain's ACT/DVE tail.
                ck, cq = chain_items("k", 0, 0), chain_items("q", 0, 0)
                for _, fn in (ck[0], cq[0], ck[1], cq[1], ck[2], cq[2]):
                    fn()

                # ---- filler schedule: block idx -> list of (kind, fn) ----
                fillers = {i: [] for i in range(8)}
                PAD = ("pad", lambda: None)
                # block order is c0-major: blocks 0-3 = (qb0..3, c0),
                # blocks 4-7 = (qb0..3, c1).  Every chain's B2 must be
                # emitted before the first QK of the block that consumes its
                # kT/qTs slice, and k(c,sb)'s B2 before QK j=4*sb of any
                # block with that c (emission order IS the dependency order;
                # QK(j) is emitted at loop index g=j-1 via the +1 lookahead).
                vs = [v_item(mc)[0] for mc in range(NKV)]
                k1, k2, k3 = (chain_items("k", 0, 1), chain_items("k", 0, 2),
                              chain_items("k", 0, 3))
                q01 = chain_items("q", 0, 1)
                # block0, EDF order (npop=2 while backlogged): k(c0,sb1)
                # parts pop by iter<=2 (QK j=4 is emitted at iter 3 via the
                # +1 lookahead), sb2 by 6, sb3 by 10; v(mc) by iter mc+3
                # (AV(j) is emitted at iter j+AVLAG); q(c0,sb1) by 14.
                fillers[0] = [k1[0], k1[1], k1[2], vs[0],
                              vs[1], vs[2], vs[3], k2[0],
                              k2[1], k2[2], vs[4], vs[5],
                              vs[6], k3[0], vs[7], k3[1],
                              k3[2], vs[8], vs[9], vs[10],
                              vs[11], q01[0], vs[12], q01[1],
                              q01[2], vs[13], vs[14], vs[15]]

                def weave(*chains):
                    return [c[i] for i in range(3) for c in chains]

                fillers[1] = weave(chain_items("q", 0, 2),
                                   chain_items("k", 1, 0),
                                   chain_items("k", 1, 1))
                fillers[2] = weave(chain_items("q", 0, 3),
                                   chain_items("q", 1, 0),
                                   chain_items("k", 1, 2))
                fillers[3] = weave(chain_items("k", 1, 3),
                                   chain_items("q", 1, 1))

                def one_chain(c):
                    return [c[0], PAD, c[1], PAD, PAD, c[2]]

                fillers[4] = one_chain(chain_items("q", 1, 2))
                fillers[5] = one_chain(chain_items("q", 1, 3))
                # out-projections: op(qb) needs attT(qb,c0) [block qb] and
                # attT(qb,c1) [block 4+qb]
                opi = {qb: [op_item(m, nb)[0]
                            for m in range(qb * 4, qb * 4 + 4)
                            for nb in range(D // SB)]
                       for qb in range(NSB - 1)}
                fillers[5] += opi[0]
                fillers[6] += opi[1]
                fillers[7] = opi[2]
                # tail: out-proj of the last q-block, split A/B: the c0
                # matmuls depend only on attT(3,c0) (ready since block 3),
                # so they pre-run into the freed qk/acc PSUM banks while
                # the last deferred AVs stream; after the last norm only
                # the c1 matmuls + evictions + DMA (spread over the three
                # idle-at-tail queues) remain.
                tail_eng = [nc.sync, nc.scalar, nc.gpsimd]
                tail_mn = [(m, nb) for m in range(12, 16)
                           for nb in range(D // SB)]
                tail_pon = {}

                def tail_phase_a():
                    for i in range(0, 6, 2):
                        t = qk.tile([P, 2, SB], f32, name="qk_t")
                        tail_pon[tail_mn[i]] = t[:, 0, :]
                        tail_pon[tail_mn[i + 1]] = t[:, 1, :]
                    for i in range(6, 8):
                        tail_pon[tail_mn[i]] = acc.tile(
                            [P, SB], f32, name="pon", tag="acc")
                    for (m, nb), pon in tail_pon.items():
                        nc.tensor.matmul(
                            pon, attT_sb[:, 0, m * P:(m + 1) * P],
                            wo_sb[:, 0, nb * SB:(nb + 1) * SB],
                            start=True, stop=False)

                def tail_phase_b():
                    for i, (m, nb) in enumerate(tail_mn):
                        pon = tail_pon[(m, nb)]
                        nc.tensor.matmul(
                            pon, attT_sb[:, 1, m * P:(m + 1) * P],
                            wo_sb[:, 1, nb * SB:(nb + 1) * SB],
                            start=False, stop=True)
                        osb = ev.tile([P, SB], f16, name="osb")
                        nc.vector.tensor_copy(osb, pon)
                        tail_eng[i % 3].dma_start(
                            out_d[m, :, nb * SB:(nb + 1) * SB], osb)

                # ---- phase 2: software-pipelined attention stream ----
                blocks = [(qb, c) for c in range(CL) for qb in range(NSB)]
                groups = [(bi, qb, c, j)
                          for bi, (qb, c) in enumerate(blocks)
                          for j in range(NKV)]
                sc_of = {}
                avs_of = {}

                def emit_qk(g):
                    bi, qb, c, j = groups[g]
                    sc2 = qk.tile([P, 2, SB], f32, name="qk_t")
                    q0 = qb * SB
                    for h in range(2):
                        po = h * HD
                        nc.tensor.matmul(
                            sc2[:, h, :],
                            kT_sb[po:po + HD, c, j * P:(j + 1) * P],
                            qTs_sb[po:po + HD, c, q0:q0 + SB],
                            start=True, stop=True,
                        )
                    sc_of[g] = sc2

                AVLAG = 6  # AV trails exp by 6 groups (9 for j<3): absorbs
                #            the norm latency of the previous block (av
                #            bufs=1) without blocking the in-order PE queue.
                ex_of = {}
                # every DVE_MOD'th group's exp runs on the DVE as a one-pass
                # Schraudolph: int16(x*1024*log2e + magic) bitcast to f16 is
                # exp(x) with ~1.8% RMS err; softmax ratios tolerate it and
                # only 1/DVE_MOD of each row's weights are approximate.  This
                # breaks the ACT 1147ns/group pacing floor.
                DVE_MOD = 2
                i16 = mybir.dt.int16
                ALU = mybir.AluOpType
                EXP_A = 1.4426950408889634 * 1024.0
                EXP_MAGIC = 15360.0 - 44.0

                def emit_exp(g):
                    sc2 = sc_of.pop(g)
                    ex2 = exp_pool.tile([P, 2, SB], f16, name="ex_t")
                    j = groups[g][3]
                    if g % DVE_MOD == DVE_MOD - 1 and j not in (0, 1, 14, 15):
                        nc.vector.tensor_scalar(
                            ex2.bitcast(i16), sc2, EXP_A, EXP_MAGIC,
                            op0=ALU.mult, op1=ALU.add)
                    else:
                        nc.scalar.activation(ex2, sc2, AF.Exp)
                    ex_of[g] = ex2

                def emit_av(g):
                    bi, qb, c, j = groups[g]
                    ex2 = ex_of.pop(g)
                    if j == 0:
                        avs_of[bi] = [
                            avp.tile([HD + 1, SB], f32, name=f"av{h}",
                                     tag=f"av{h}") for h in range(2)]
                    for h in range(2):
                        nc.tensor.matmul(
                            avs_of[bi][h],
                            vaug_sb[:, j, c * 2 + h, :],
                            ex2[:, h, :],
                            start=(j == 0), stop=(j == NKV - 1),
                        )
                    if j == NKV - 1:
                        avs = avs_of.pop(bi)
                        q0 = qb * SB
                        for h in range(2):
                            po = h * HD
                            # evict the whole accumulator to SBUF first: the
                            # PSUM banks free after ONE copy instead of the
                            # full copy->recip->bcast->mul chain, so the next
                            # block's AV j=0 (in-order PE queue) isn't stalled
                            # behind the norm.
                            avsb = ev.tile([HD + 1, SB], f32, name="avsb",
                                           bufs=3)
                            nc.scalar.copy(avsb, avs[h])
                            # plain copy handles the partition shift (64->0);
                            # partition-shifted custom-DVE ops are not
                            # trustworthy on HW.
                            srow = ev.tile([1, SB], f32, name="srow")
                            nc.vector.tensor_copy(srow, avsb[HD:HD + 1, :])
                            if bi == len(blocks) - 1:
                                # tail-latency path: broadcast the recip row
                                # on the PE (onesel-style) instead of the
                                # ~1us gpsimd partition_broadcast
                                rr16 = ev.tile([1, SB], f16, name="rr16")
                                recip(nc, rr16, srow)
                                # reuse this head's own (just-evicted) av
                                # bank -- the acc pool is holding tail c0
                                # partials at this point
                                rbp = avp.tile([HD + 1, SB], f32,
                                               name=f"av{h}", tag=f"av{h}")
                                nc.tensor.matmul(rbp[0:HD, :],
                                                 onesel[0:1, 0:HD],
                                                 rr16, start=True, stop=True)
                                nc.vector.tensor_mul(
                                    attT_sb[po:po + HD, c, q0:q0 + SB],
                                    avsb[0:HD, :], rbp[0:HD, :])
                            else:
                                rrow = ev.tile([1, SB], f32, name="rrow")
                                recip(nc, rrow, srow)
                                rbc = ev.tile([HD, SB], f32, name="rbc")
                                nc.gpsimd.partition_broadcast(
                                    rbc, rrow[0:1, :], HD)
                                nc.vector.tensor_mul(
                                    attT_sb[po:po + HD, c, q0:q0 + SB],
                                    avsb[0:HD, :], rbc)

                emit_qk(0)
                NG = len(groups)
                # j<3 AVs are deferred 3 extra slots: their block's avs banks
                # are still being freed by the previous block's eviction.
                pend_av = {}
                for g in range(NG):
                    j = groups[g][3]
                    pend_av.setdefault(g + AVLAG + (3 if j < 3 else 0),
                                       []).append(g)
                for g in range(NG + AVLAG + 3):
                    if g + 1 < NG:
                        emit_qk(g + 1)
                    if g < NG:
                        emit_exp(g)
                    if g == NG + 1:
                        # last exp has been emitted; qk banks are free after
                        # it drains -- pre-run the tail c0 matmuls
                        tail_phase_a()
                    for g2 in pend_av.pop(g, []):
                        emit_av(g2)
                    if g < NG:
                        bi, qb, c, j = groups[g]
                        # filler items per kv chunk (ops only once attT of
                        # the previous qb has had time to normalize); pop 2
                        # when the remaining slots would not drain the list
                        fl = fillers[bi]
                        npop = 1
                        if len(fl) > NKV - j:
                            npop = 2
                        minj = {"op": 6}
                        for _ in range(npop):
                            if fl and j >= minj.get(fl[0][0], 0):
                                fl.pop(0)[1]()

                # tail: finish the out-projection of the last q-block
                tail_phase_b()

    nc.compile()
    return nc


def _prepare_core_inputs(inputs):
    """Fold LN centering/gain into weights; shard per core; cast fp16."""
    q = np.asarray(inputs["query"], np.float32)
    q_w = np.asarray(inputs["q_w"], np.float64)
    k_w = np.asarray(inputs["k_w"], np.float64)
    v_w = np.asarray(inputs["v_w"], np.float32)
    o_w = np.asarray(inputs["o_w"], np.float32)
    q_b = np.asarray(inputs["q_b"], np.float64)
    k_b = np.asarray(inputs["k_b"], np.float64)
    v_b = np.asarray(inputs["v_b"], np.float32)
    q_g = np.asarray(inputs["q_ln_g"], np.float64)
    k_g = np.asarray(inputs["k_ln_g"], np.float64)

    def fold(w, b, g):
        # per head block (64 out-dims): center across the block, scale by g
        w = w.reshape(H, HD, D)
        w = (w - w.mean(axis=1, keepdims=True)) * g[None, :, None]
        b = b.reshape(H, HD)
        b = (b - b.mean(axis=1, keepdims=True)) * g[None, :]
        return w.reshape(D, D), b.reshape(D).astype(np.float32)

    wq_f, qb_f = fold(q_w, q_b, q_g)
    wk_f, kb_f = fold(k_w, k_b, k_g)

    def stat_w(g):
        # w_dd = 1/(64*g_d^2), laid out [CL, P, 33] block-diagonal per c-half
        # (head0 -> col 0, head1 -> col 32: partition-32-aligned outputs)
        w = np.zeros((CL, P, STW), np.float64)
        for c in range(CL):
            for h in range(2):
                w[c, h * HD:(h + 1) * HD, 32 * h] = 1.0 / (HD * g[:HD] ** 2)
        return w.astype(np.float16)

    wsq = stat_w(np.asarray(inputs["q_ln_g"], np.float64))
    wsk = stat_w(np.asarray(inputs["k_ln_g"], np.float64))

    in_maps = []
    for c in range(NCORES):
        b, g = divmod(c, GPC)
        rows = slice(g * DL, (g + 1) * DL)
        in_maps.append({
            "xT": np.ascontiguousarray(q[b].T).reshape(KC, P, S).astype(np.float16),
            "wqT": np.ascontiguousarray(wq_f[rows].T).reshape(KC, P, DL).astype(np.float16),
            "wkT": np.ascontiguousarray(wk_f[rows].T).reshape(KC, P, DL).astype(np.float16),
            "wvT": np.ascontiguousarray(v_w[rows].T).reshape(KC, P, DL).astype(np.float16),
            "woT": np.ascontiguousarray(o_w[:, rows].T).reshape(CL, P, D).astype(np.float16),
            "qb": np.ascontiguousarray(qb_f[rows]).reshape(CL, P, 1),
            "kb": np.ascontiguousarray(kb_f[rows]).reshape(CL, P, 1),
            "vb": np.ascontiguousarray(v_b[rows]).reshape(1, DL),
            "wsq": wsq,
            "wsk": wsk,
        })
    return in_maps


def _install_ntff_shim():
    """The agent image's antenv lacks axon_hooks; recreate it so
    run_bass_kernel_spmd(trace=True) can capture NTFF profiles."""
    import types

    try:
        import antenv.axon_hooks  # noqa: F401
        return
    except ImportError:
        pass
    import antenv
    mod = types.ModuleType("antenv.axon_hooks")
    mod._hook = None
    mod.set_axon_ntff_profile_hook = lambda h: setattr(mod, "_hook", h)
    mod.get_axon_ntff_profile_hook = lambda: mod._hook
    sys.modules["antenv.axon_hooks"] = mod
    antenv.axon_hooks = mod
    try:
        from trn_agent_boot.trn_boot import _ntff_profile_via_ctypes
        hook = _ntff_profile_via_ctypes("/opt/axon/libaxon_pjrt.so")
        if hook is not None:
            mod.set_axon_ntff_profile_hook(hook)
    except Exception as e:
        print(f"ntff shim: hook install failed: {e}", file=sys.stderr)


def kernel(**inputs):
    import concourse.bass_utils as bass_utils
    from concourse.bass_utils import run_bass_kernel_spmd

    if "nc" not in _CACHE:
        _CACHE["nc"] = _build_nc()
    nc = _CACHE["nc"]

    in_maps = _prepare_core_inputs(inputs)
    trace = os.environ.get("TRNK_TRACE", "0") == "1"
    if trace:
        _install_ntff_shim()
        # no S3 in this container; keep artifacts local
        bass_utils.upload_artifacts = lambda d: d
    res = run_bass_kernel_spmd(nc, in_maps, core_ids=list(range(NCORES)),
                               trace=trace)
    _CACHE["last_results"] = res

    o_b = np.asarray(inputs["o_b"], np.float32)
    out = np.zeros((B, S, D), np.float32)
    for c in range(NCORES):
        b = c // GPC
        out[b] += res.results[c]["out"].reshape(S, D).astype(np.float32)
    out += o_b[None, None, :]
    return out


if __name__ == "__main__":
    # smoke test against random inputs (no reference available standalone)
    rng = np.random.default_rng(0)
    ins = {
        "query": rng.standard_normal((B, S, D)).astype(np.float32),
        "q_w": (rng.standard_normal((D, D)) * 0.03).astype(np.float32),
        "q_b": np.zeros(D, np.float32),
        "k_w": (rng.standard_normal((D, D)) * 0.03).astype(np.float32),
        "k_b": np.zeros(D, np.float32),
        "v_w": (rng.standard_normal((D, D)) * 0.03).astype(np.float32),
        "v_b": np.zeros(D, np.float32),
        "o_w": (rng.standard_normal((D, D)) * 0.03).astype(np.float32),
        "o_b": np.zeros(D, np.float32),
        "q_ln_g": np.ones(HD, np.float32),
        "q_ln_b": np.zeros(HD, np.float32),
        "k_ln_g": np.ones(HD, np.float32),
        "k_ln_b": np.zeros(HD, np.float32),
    }
    out = kernel(**ins)
    print("out", out.shape, out.dtype, float(np.abs(out).max()))
